# revision 45
# baseline (speedup 1.0000x reference)
"""Trainium2 Bass kernel for additive (Bahdanau) attention — fp8 DoubleRow.

reference:
    proj_f = features @ W1_w + W1_b          # [B, L, ATT]
    proj_h = (hidden @ W2_w + W2_b)[:, None] # [B, 1, ATT]
    scores = tanh(proj_f + proj_h) @ V_w + V_b   # [B, L]
    alpha  = softmax(scores, axis=1)
    context = einsum('bl,ble->be', alpha, features)
    returns (alpha, context)

Sharding: data-parallel over batch B=64 across 8 cores (8 examples/core).
Weights replicated. No collectives.

ALL layout/quantization work happens on the host.  Features ship twice —
natural bf16 (context matmul) and transposed fp8 x32 in the exact SBUF
layout the DoubleRow GEMM wants — so the device does zero transposes and
zero casts: every PE cycle is GEMM / V-dot / context / tiny softmax
glue.  W1 ships as fp8 x64 DoubleRow pairs, W2/hidden as fp8 (x64/x32,
descaled 1/2048 after the proj_h matmul), bias/V pre-transposed.  Few,
large dma_starts throughout (a dma_start costs ~0.7-1 us of serialized
sync-engine issue + one of 8 DMA semaphore lanes; one InstDMACopy
already spreads across all 16 SDMA engines), ordered so the transfers
gating the first GEMM block get the early bandwidth to themselves.

Per-core algorithm (X = 8 examples):
  - main GEMM [a,l] and V-dot run in fp8e4 DoubleRow (K=256/matmul);
    tanh descales via its scale arg, exp via 1/64 (softmax is
    shift-invariant so V_b is dropped).
  - scores accumulate in PSUM [1,512] via V-dot matmuls trailing the
    tanh by two blocks; no-max softmax straight from score PSUM.
  - context: three e-splits (384/384/256) run as CONCURRENT M=1
    accumulation chains in PE column groups 0/32/64 against the natural
    bf16 tiles, using the UNNORMALIZED exp; 1/sum folds into the final
    PSUM->SBUF copies.  Deferred into the next example's early blocks.
  - proj_h: computed once as out[x,a] (hT stationary, two N=512
    streams), transposed back per a-block, bias added.
  - last example: its h0 alpha-transposes are pulled off the tail's
    critical path (emitted right after the h0 exp).
"""

import numpy as np
import ml_dtypes

B, L, ENC, DEC, ATT = 64, 1024, 1024, 1024, 1024
N_CORES = 8
X = B // N_CORES  # examples per core
P = 128
NE = ENC // P  # 8
NA = ATT // P  # 8
ND = DEC // P  # 8
LH = 512       # free-dim half for fp32 PSUM bank
NL = L // LH   # 2

FSCALE = 32.0                      # feature scale into fp8
WSCALE = 64.0                      # W1 / V scale into fp8
PSCALE = 1.0 / (FSCALE * WSCALE)   # descale inside tanh
SSCALE = 1.0 / WSCALE              # descale inside exp

_CACHE = {}


def _build():
    import concourse.bacc as bacc
    import concourse.mybir as mybir
    import concourse.tile as tile

    f32, bf16 = mybir.dt.float32, mybir.dt.bfloat16
    fp8 = mybir.dt.float8e4
    Tanh = mybir.ActivationFunctionType.Tanh
    Exp = mybir.ActivationFunctionType.Exp
    DR = mybir.MatmulPerfMode.DoubleRow

    nc = bacc.Bacc("TRN2", target_bir_lowering=False, debug=False, num_devices=N_CORES)

    feats = nc.declare_dram_parameter("features", [X, L, ENC], bf16, isOutput=False)
    ft8in = nc.declare_dram_parameter("ft8in", [X, P, NE * L], fp8, isOutput=False)
    w1d8 = nc.declare_dram_parameter("w1d8", [P, NE // 2, 2, ATT], fp8, isOutput=False)
    w2bf = nc.declare_dram_parameter("w2bf", [P, ND, ATT], fp8, isOutput=False)
    hT8 = nc.declare_dram_parameter("hT8", [P, ND, X], fp8, isOutput=False)
    bT8 = nc.declare_dram_parameter("bT8", [P, NA], f32, isOutput=False)
    vwd8 = nc.declare_dram_parameter("vwd8", [P, 2, 16], fp8, isOutput=False)
    alpha_o = nc.declare_dram_parameter("alpha", [X, L], f32, isOutput=True)
    ctx_o = nc.declare_dram_parameter("context", [X, ENC], f32, isOutput=True)

    eye_dram = nc.inline_tensor(np.eye(P, dtype=np.float32), "eye128")

    with tile.TileContext(nc) as tc:
        with (
            tc.tile_pool(name="const", bufs=1) as const,
            tc.tile_pool(name="fb", bufs=8) as fbp,
            tc.tile_pool(name="f8", bufs=4) as f8p,
            tc.tile_pool(name="mm", bufs=3, space="PSUM") as psum,
            tc.tile_pool(name="sc", bufs=3, space="PSUM") as spsum,
            tc.tile_pool(name="tp", bufs=2, space="PSUM") as tpsum,
            tc.tile_pool(name="tb", bufs=6) as tp,
            tc.tile_pool(name="al", bufs=2) as alp,
            tc.tile_pool(name="ms", bufs=1) as ms,
        ):
            # ---------------- per-example staging helpers ----------------
            fb_map = {}     # x -> {c: bf16 natural chunk view [P, ENC]}
            fbh_map = {}    # (x, half) -> bf16 half tile [P, 4, ENC]
            f8_map = {}     # x -> ft8 tile  [P, NE*L] fp8 (features x32)

            def emit_dma_half(x, half):
                # natural bf16 features (context matmul operand): one 1 MB
                # dma_start per half
                fbh = fbp.tile([P, 4, ENC], bf16, tag="fb", name=f"fb{x}_{half}")
                fbh_map[(x, half)] = fbh
                for c in range(4):
                    fb_map.setdefault(x, {})[4 * half + c] = fbh[:, c, :]
                src = feats[x, 512 * half : 512 * (half + 1), :].rearrange(
                    "(c p) e -> p c e", c=4
                )
                nc.sync.dma_start(fbh[:], src)

            def emit_dma_ft8(x):
                # transposed fp8 x32 features in final SBUF layout: one 1 MB
                # dma_start per example
                f8_map[x] = f8p.tile([P, NE * L], fp8, tag="f8", name=f"f8{x}")
                nc.sync.dma_start(f8_map[x][:], ft8in[x, :, :])

            # ---------------- prologue ----------------
            # critical first: ft8(x0) + W1 gate the first GEMM block, W2/hT/bT
            # gate proj_h (needed by the first tanh).  Everything else is
            # issued from inside the loop so the critical transfers get the
            # early DMA bandwidth to themselves.
            emit_dma_ft8(0)
            w1all = const.tile([P, NE // 2, 2, ATT], fp8, tag="w1all")
            nc.sync.dma_start(w1all[:], w1d8[:, :, :, :])
            w2all = ms.tile([P, ND, ATT], fp8, tag="w2all")
            nc.sync.dma_start(w2all[:], w2bf[:, :, :])
            eye = const.tile([P, P], f32, tag="eye")
            nc.sync.dma_start(eye[:], eye_dram[:, :])
            hTb = ms.tile([P, ND, X], fp8, tag="hTb")
            nc.sync.dma_start(hTb[:], hT8[:, :, :])
            bT = ms.tile([P, NA], f32, tag="bT")
            nc.sync.dma_start(bT[:], bT8[:, :])
            vwd = ms.tile([P, 2, 16], fp8, tag="vwd")
            nc.sync.dma_start(vwd[:], vwd8[:, :, :])

            # proj_h + bias, transposed into phb[p, a, x].  Computed as
            # out[x, a] with hT stationary (two N=512 streams, LDWs hidden),
            # then transposed back in 8 [8,128] chunks.
            phb = ms.tile([P, NA, X], f32, tag="phb")
            ph_xa = ms.tile([X, ATT], f32, tag="ph_xa")
            for ah in range(2):
                ph_ps = psum.tile([X, LH], f32, tag="mm", name=f"phps{ah}")
                for e in range(ND):
                    nc.tensor.matmul(
                        ph_ps[:],
                        hTb[:, e, :],
                        w2all[:, e, LH * ah : LH * (ah + 1)],
                        start=(e == 0),
                        stop=(e == ND - 1),
                    )
                nc.vector.tensor_scalar_mul(
                    ph_xa[:, LH * ah : LH * (ah + 1)], ph_ps[:], 1.0 / 2048.0
                )
            for a in range(NA):
                ph_tp = tpsum.tile([P, X], f32, tag="tp", name=f"phtp{a}")
                nc.tensor.transpose(
                    ph_tp[:], ph_xa[:, P * a : P * (a + 1)], eye[0:X, 0:X]
                )
                nc.vector.tensor_scalar_add(phb[:, a, :], ph_tp[:], bT[:, a : a + 1])

            # ---------------- main per-example pipeline ----------------
            # feature-DMA issue schedule: (x, b) -> action.  ft8(x) is needed
            # at x's block 0; fb(x) only by x's context matmuls at (x+1) b5.
            dma_sched = {(0, 1): [("ft8", 1)], (0, 3): [("ft8", 2)]}
            for x in range(1, X):
                if x + 2 < X:
                    dma_sched[(x, 0)] = [("ft8", x + 2)]
            for x in range(X):
                dma_sched.setdefault((x, 4), []).append(("fb", x, 0))
                dma_sched.setdefault((x, 8), []).append(("fb", x, 1))

            def run_sched(x, b):
                for act in dma_sched.get((x, b), []):
                    if act[0] == "ft8":
                        emit_dma_ft8(act[1])
                    else:
                        emit_dma_half(act[1], act[2])

            pending = []

            def flush_pending(cur_b=10**6):
                keep = []
                for sc_ap, j, tb_ap, b_emit in pending:
                    if b_emit <= cur_b - 2:
                        nc.tensor.matmul(
                            sc_ap,
                            vwd[:, :, j : j + 1],
                            tb_ap,
                            start=(j == 0),
                            stop=(j == 3),
                            perf_mode=DR,
                        )
                    else:
                        keep.append((sc_ap, j, tb_ap, b_emit))
                pending[:] = keep

            pending_ctx = []

            def flush_ctx(n):
                for _ in range(min(n, len(pending_ctx))):
                    pending_ctx.pop(0)()

            for x in range(X):
                run_sched(x, 0)

                ft8v = f8_map[x].rearrange("p (e l) -> p e l", e=NE)
                sc_h = {}
                tb3 = None
                tail_tps = None
                for b in range(16):
                    lh, a = divmod(b, 8)
                    j, i = divmod(a, 2)
                    if a == 0:
                        sc_h[lh] = spsum.tile([1, LH], f32, tag="sc", name=f"sch{x}_{lh}")
                    if i == 0:
                        tb3 = tp.tile([P, 2, LH], fp8, tag="tb")
                    pp = psum.tile([P, LH], f32, tag="mm")
                    for q in range(4):
                        nc.tensor.matmul(
                            pp[:],
                            w1all[:, q, :, P * a : P * (a + 1)],
                            ft8v[:, 2 * q : 2 * q + 2, LH * lh : LH * (lh + 1)],
                            start=(q == 0),
                            stop=(q == 3),
                            perf_mode=DR,
                        )
                        if q == 1:
                            flush_pending(b)

                    nc.scalar.activation(
                        tb3[:, i, :], pp[:], Tanh,
                        bias=phb[:, a, x : x + 1], scale=PSCALE,
                    )
                    if i == 1:
                        pending.append((sc_h[lh][:], j, tb3[:], b))

                    # deferred context matmuls of example x-1, behind the
                    # softmax latency
                    if 4 <= b < 8:
                        flush_ctx(1)
                    if b > 0:
                        run_sched(x, b)

                    if b == 9:
                        # scores half 0 is complete (trail-2 flush at b9):
                        # unnormalized exp straight from PSUM
                        esb = alp.tile([1, L], f32, tag="esb", name=f"esb{x}")
                        ssum0 = alp.tile([1, 1], f32, tag="ssum0")
                        nc.scalar.activation(
                            esb[:, 0:LH], sc_h[0][:], Exp, scale=SSCALE,
                            accum_out=ssum0[:],
                        )
                    if b == 11 and x == X - 1:
                        # last example: pull the h0 alpha-transposes off the
                        # tail's critical path (esb h0 is ready after b9)
                        tail_tps = tpsum.tile([P, NE], f32, tag="tp", name="tail_tps")
                        for lc in range(4):
                            nc.tensor.transpose(
                                tail_tps[:, lc : lc + 1],
                                esb[:, P * lc : P * (lc + 1)],
                                eye[0:1, 0:1],
                            )

                flush_pending()

                # finish the no-max softmax: exp of half 1, sum, reciprocal
                ssum1 = alp.tile([1, 1], f32, tag="ssum1")
                nc.scalar.activation(
                    esb[:, LH:L], sc_h[1][:], Exp, scale=SSCALE,
                    accum_out=ssum1[:],
                )
                ssum = alp.tile([1, 1], f32, tag="ssum")
                nc.vector.tensor_add(ssum[:], ssum0[:], ssum1[:])
                rinv = alp.tile([1, 1], f32, tag="rinv")
                nc.vector.reciprocal(rinv[:], ssum[:])
                a32 = alp.tile([1, L], f32, tag="scores", name=f"a32_{x}")
                nc.vector.tensor_scalar_mul(a32[:], esb[:], rinv[:])
                nc.sync.dma_start(alpha_o[x, :], a32[:])

                # context on PE against the natural bf16 tiles, deferred into
                # x+1's early blocks.  Uses the UNNORMALIZED exp (esb) so it
                # only waits on the exps, not on a32; 1/sum lands in the final
                # PSUM->SBUF copies.  The two e-halves run as CONCURRENT
                # accumulation chains in PE column groups 0 and 32 (M=1 each;
                # base partition 96 is rejected by bass, so 2-way is the max
                # usable col-tiling here).
                def make_ctx(x, esb, rinv, tail_tps):
                    fb_x = fb_map[x]
                    alT = alp.tile([P, NE], bf16, tag="alT", name=f"alT{x}")
                    ctr2 = alp.tile([1, ENC], f32, tag="ctr2", name=f"ctr2_{x}")
                    state = {}

                    def stage0():
                        if tail_tps is None:
                            tps_a = tpsum.tile([P, NE], f32, tag="tp", name=f"tpsa{x}")
                            lo = 0
                        else:
                            tps_a = tail_tps
                            lo = 4
                        for lc in range(lo, NE):
                            nc.tensor.transpose(
                                tps_a[:, lc : lc + 1], esb[:, P * lc : P * (lc + 1)],
                                eye[0:1, 0:1],
                            )
                        nc.vector.tensor_copy(alT[:], tps_a[:])

                    SPLITS = [(0, 384), (384, 768), (768, 1024)]

                    def mms():
                        cps = psum.tile([65, 384], f32, tag="mm", name=f"cps{x}")
                        state["cps"] = cps
                        for lc in range(NE):
                            for eq, (lo, hi) in enumerate(SPLITS):
                                nc.tensor.matmul(
                                    cps[32 * eq : 32 * eq + 1, 0 : hi - lo],
                                    alT[:, lc : lc + 1],
                                    fb_x[lc][:, lo:hi],
                                    start=(lc == 0),
                                    stop=(lc == NE - 1),
                                )

                    def fin():
                        cps = state["cps"]
                        for eq, (lo, hi) in enumerate(SPLITS):
                            nc.vector.tensor_scalar_mul(
                                ctr2[:, lo:hi],
                                cps[32 * eq : 32 * eq + 1, 0 : hi - lo],
                                rinv[:],
                            )
                        nc.sync.dma_start(ctx_o[x, :], ctr2[:])

                    return [stage0, mms, fin]

                pending_ctx.extend(make_ctx(x, esb, rinv, tail_tps))
                if x == X - 1:
                    flush_ctx(len(pending_ctx))

    nc.compile()
    return nc


def _prep_weights(W1_w, W1_b, W2_w, W2_b, V_w):
    fp8 = ml_dtypes.float8_e4m3
    bf16 = ml_dtypes.bfloat16
    # w1d8[p, q, i, m] = 64 * W1[(2q+i)*128 + p, m]
    w1d8 = np.ascontiguousarray(
        (W1_w * WSCALE).reshape(NE // 2, 2, P, ATT).transpose(2, 0, 1, 3).astype(fp8)
    )
    # w2bf[p, e, m] = 64 * W2[128e + p, m]  (fp8)
    w2bf = np.ascontiguousarray(
        (W2_w * WSCALE).reshape(ND, P, ATT).transpose(1, 0, 2).astype(fp8)
    )
    # bT8[p, a] = W1_b[128a+p] + W2_b[128a+p]
    bT8 = np.ascontiguousarray((W1_b + W2_b).reshape(NA, P).T.astype(np.float32))
    # vwd8[p, i, j] = 64 * V[(2j+i)*128 + p], j < 4, padded to 16
    vwd8 = np.zeros((P, 2, 16), dtype=fp8)
    vwd8[:, :, 0:4] = (V_w * WSCALE).reshape(4, 2, P).transpose(2, 1, 0).astype(fp8)
    return w1d8, w2bf, bT8, np.ascontiguousarray(vwd8)


def kernel(features, hidden_state, W1_w, W1_b, W2_w, W2_b, V_w, V_b):
    from concourse.bass_utils import run_bass_kernel_spmd

    if "nc" not in _CACHE:
        _CACHE["nc"] = _build()
    nc = _CACHE["nc"]

    features = np.asarray(features, dtype=np.float32).astype(ml_dtypes.bfloat16)
    hidden_state = np.asarray(hidden_state, dtype=np.float32)
    W1_w = np.asarray(W1_w, dtype=np.float32)
    W1_b = np.asarray(W1_b, dtype=np.float32)
    W2_w = np.asarray(W2_w, dtype=np.float32)
    W2_b = np.asarray(W2_b, dtype=np.float32)
    V_w = np.asarray(V_w, dtype=np.float32)

    w1d8, w2bf, bT8, vwd8 = _prep_weights(W1_w, W1_b, W2_w, W2_b, V_w)
    fp8 = ml_dtypes.float8_e4m3

    in_maps = []
    for c in range(N_CORES):
        h = hidden_state[c * X : (c + 1) * X]
        # hT8[p, cc, x] = h[x, 128cc+p]
        hT8 = np.ascontiguousarray(
            (h.reshape(X, ND, P).transpose(2, 1, 0) * FSCALE).astype(fp8)
        )
        fshard = np.ascontiguousarray(features[c * X : (c + 1) * X])
        # ft8in[x, p, e*L + l] = fp8(32 * f[x, l, 128e+p])
        ft8 = np.ascontiguousarray(
            (fshard.reshape(X, L, NE, P).transpose(0, 3, 2, 1)
             .astype(np.float32) * FSCALE).astype(fp8).reshape(X, P, NE * L)
        )
        in_maps.append(
            {
                "features": fshard,
                "ft8in": ft8,
                "hT8": hT8,
                "w1d8": w1d8,
                "w2bf": w2bf,
                "bT8": bT8,
                "vwd8": vwd8,
            }
        )

    res = run_bass_kernel_spmd(nc, in_maps, list(range(N_CORES)), **_CACHE.get("run_kwargs", {}))
    _CACHE["last_result"] = res
    alpha = np.concatenate([res.results[c]["alpha"] for c in range(N_CORES)], axis=0)
    context = np.concatenate([res.results[c]["context"] for c in range(N_CORES)], axis=0)
    return alpha, context
